# revision 1
# baseline (speedup 1.0000x reference)
"""MoE top-2/8 expert-parallel kernel for TRN2 (8 cores).

Per-core (core c == expert c) pipeline:
  1. Router on own 512-token slice of x, in split-bf16 (3-term) for fp32-level
     accuracy: logits_slice = xh@rkh + xl@rkh + xh@rkl.
  2. AllGather logits (tiny) + AllGather bf16(x) slices (gather source).
  3. Top-2 per token via DVE max8/max_index; weights w1 = sigmoid((E1-E2)/Z);
     per-expert compaction via triangular-matmul cumsum -> slot per token;
     token-id list built with an indirect scatter (pads -> trash slots).
  4. dma_gather(transpose=True): xeT [128, D/128, 384] x3 bf16 gathered+transposed.
  5. gate/up matmuls (bf16, fp32 accum) -> silu*up -> fuse bf16
     down matmul -> scale by per-slot gate weight -> dma_scatter_add into a
     dense [T(+trash), D] bf16 partial.
  6. ReduceScatter(bf16) over 8 cores -> own 512-token slice -> fp32 out.
"""

import numpy as np
import concourse.bass as bass


class _StageDone(Exception):
    pass


import concourse.mybir as mybir
import concourse.tile as tile
from concourse import bacc
from concourse.bass import IndirectOffsetOnAxis
from concourse.masks import make_identity, make_upper_triangular

P = 128
T, D, F, E = 4096, 2048, 1024, 8
TS = T // 8          # tokens per core slice
GC = 384             # gather chunk (3 chunks = C)
C = 3 * GC           # 1152 per-expert token capacity (measured max 1058)
NTRASH = 64          # trash rows / slots
dt = mybir.dt
AF = mybir.ActivationFunctionType
ALU = mybir.AluOpType

TO = T // P   # 32 token columns (t = o*128 + p)
SO = TS // P  # 4 token columns per slice
KO = D // P   # 16 contraction tiles over D
FO = F // P   # 8 f-tiles
CM = C // P   # 9 slot tiles
C16 = C // 16           # 72
CI16 = (C + NTRASH) // 16  # 76


def build(n_cores: int = 8, repeat: int = 1, rs_split: bool = True, fuse_ag: bool = False):
    TS = T // n_cores
    SO = TS // P
    nc = bacc.Bacc("TRN2", target_bir_lowering=False, debug=False,
                   num_devices=n_cores)

    xs = nc.dram_tensor("xs", [TS, D], dt.float32, kind="ExternalInput")
    rk = nc.dram_tensor("rk", [D, E], dt.float32, kind="ExternalInput")
    wg = nc.dram_tensor("wg", [D, F], dt.float32, kind="ExternalInput")
    wu = nc.dram_tensor("wu", [D, F], dt.float32, kind="ExternalInput")
    wd = nc.dram_tensor("wd", [F, D], dt.float32, kind="ExternalInput")
    eid = nc.dram_tensor("eid", [P, 1], dt.float32, kind="ExternalInput")
    out = nc.dram_tensor("out", [TS, D], dt.float32, kind="ExternalOutput")

    with tile.TileContext(nc) as tc:
        with (
            tc.tile_pool(name="dram", bufs=1, space="DRAM") as dram,
            tc.tile_pool(name="consts", bufs=1) as consts,
            tc.tile_pool(name="wpool", bufs=1) as wpool,
            tc.tile_pool(name="main", bufs=1) as main,
        ):
          for _rep in range(repeat):
            # ---------------- DRAM scratch ----------------
            xh_slice = dram.tile([TS, D], dt.bfloat16)
            lg_slice = dram.tile([TS, E], dt.float32)
            xbf_full = dram.tile([T, D], dt.bfloat16, addr_space="Shared")
            lg_full = dram.tile([T, E], dt.float32, addr_space="Shared")
            slotd = dram.tile([T], dt.int16)
            ilist2 = dram.tile([C + NTRASH, 64], dt.float32)
            tboth = dram.tile([2 * C], dt.int16)
            acc = dram.tile([T + NTRASH, D], dt.bfloat16)
            rs_out = dram.tile([TS, D], dt.bfloat16)

            # ---------------- constants ----------------
            ident_bf = consts.tile([P, P], dt.bfloat16)
            make_identity(nc, ident_bf[:])
            ident_f32 = consts.tile([P, P], dt.float32)
            make_identity(nc, ident_f32[:])
            triu_bf = consts.tile([P, P], dt.bfloat16)
            make_upper_triangular(nc, triu_bf[:], val=1.0, diag=True)
            eid_sb = consts.tile([P, 1], dt.float32)
            nc.sync.dma_start(eid_sb[:], eid[:])

            zero_sb = consts.tile([P, D], dt.bfloat16)
            nc.vector.memset(zero_sb[:], 0.0)
            zero_f32 = consts.tile([NTRASH, 1], dt.float32)
            nc.vector.memset(zero_f32[:], 0.0)
            ones_bf = consts.tile([P, 1], dt.bfloat16)
            nc.vector.memset(ones_bf[:], 1.0)

            # -------- weights: cast-DMA fp32 -> bf16 SBUF ----------------
            wg_sb = wpool.tile([P, KO, F], dt.bfloat16)
            wu_sb = wpool.tile([P, KO, F], dt.bfloat16)
            wg_r = wg[:].rearrange("(ko p) f -> p ko f", p=P)
            wu_r = wu[:].rearrange("(ko p) f -> p ko f", p=P)
            wd_r = wd[:].rearrange("(fo p) d -> p fo d", p=P)

            # ---------------- router phase (blocks of 512 tokens) --------
            BO = 4  # 512-token blocks
            with tc.tile_pool(name="route", bufs=1) as route, \
                 tc.tile_pool(name="routeb", bufs=1) as routeb, \
                 tc.tile_pool(name="ps_route", bufs=2, space="PSUM") as psr:
                rk_f32 = route.tile([P, KO, E], dt.float32)
                nc.sync.dma_start(rk_f32[:],
                                  rk[:].rearrange("(ko p) e -> p ko e", p=P))
                rkh = route.tile([P, KO, E], dt.bfloat16)
                rkl = route.tile([P, KO, E], dt.bfloat16)
                rkh32 = route.tile([P, KO, E], dt.float32)
                nc.vector.tensor_copy(rkh[:], rk_f32[:])
                nc.vector.tensor_copy(rkh32[:], rkh[:])
                nc.vector.tensor_tensor(rkl[:], rk_f32[:], rkh32[:], ALU.subtract)

                lg_sb = route.tile([P, SO, E], dt.float32)
                xs_r = xs[:].rearrange("(o p) d -> p o d", p=P)
                xhs_r = xh_slice[:].rearrange("(o p) d -> p o d", p=P)
                for tb in range(SO // BO):
                    ob0 = tb * BO
                    xh_sb = routeb.tile([P, BO, D], dt.bfloat16, tag="xh_sb",
                                        name="xh_sb")
                    xl_sb = routeb.tile([P, BO, D], dt.bfloat16, tag="xl_sb",
                                        name="xl_sb")
                    for o in range(BO):
                        xs_o = routeb.tile([P, D], dt.float32, tag="xs_o",
                                           name="xs_o")
                        xh32 = routeb.tile([P, D], dt.float32, tag="xh32",
                                           name="xh32")
                        nc.sync.dma_start(xs_o[:], xs_r[:, ob0 + o])
                        nc.vector.tensor_copy(xh_sb[:, o], xs_o[:])
                        nc.vector.tensor_copy(xh32[:], xh_sb[:, o])
                        nc.vector.tensor_tensor(xl_sb[:, o], xs_o[:],
                                                xh32[:], ALU.subtract)
                    nc.sync.dma_start(xhs_r[:, ob0:ob0 + BO], xh_sb[:])
                    if tb == SO // BO - 1:
                        # issue the big AllGather now: it overlaps the router
                        # matmuls + routing math below
                        if n_cores > 1:
                            nc.gpsimd.collective_compute(
                                "AllGather", ALU.bypass,
                                ins=[xh_slice[:].opt()],
                                outs=[xbf_full[:].opt()],
                                replica_groups=[list(range(n_cores))])
                        else:
                            nc.sync.dma_start(xbf_full[:], xh_slice[:])

                    xhT = routeb.tile([P, KO, BO * P], dt.bfloat16, tag="xhT",
                                      name="xhT")
                    xlT = routeb.tile([P, KO, BO * P], dt.bfloat16, tag="xlT",
                                      name="xlT")
                    for src, dstT in ((xh_sb, xhT), (xl_sb, xlT)):
                        for ko in range(KO):
                            for o in range(BO):
                                pt = psr.tile([P, P], dt.bfloat16, tag="tp",
                                              name="pt")
                                nc.tensor.transpose(
                                    pt[:], src[:, o, ko * P:(ko + 1) * P],
                                    ident_bf[:])
                                nc.vector.tensor_copy(
                                    dstT[:, ko, o * P:(o + 1) * P], pt[:])

                    ps_l = psr.tile([E, BO * P], dt.float32, tag="psl",
                                    name="ps_l")
                    steps = []
                    for ko in range(KO):
                        steps.append((rkh[:, ko], xhT[:, ko]))
                        steps.append((rkl[:, ko], xhT[:, ko]))
                        steps.append((rkh[:, ko], xlT[:, ko]))
                    for i, (lhsT, rhs) in enumerate(steps):
                        nc.tensor.matmul(ps_l[:], lhsT, rhs, start=(i == 0),
                                         stop=(i == len(steps) - 1))
                    lgT_sb = routeb.tile([E, BO * P], dt.float32, tag="lgT",
                                         name="lgT_sb")
                    nc.vector.tensor_copy(lgT_sb[:], ps_l[:])
                    for o in range(BO):
                        pt2 = psr.tile([P, E], dt.float32, tag="tp2", name="pt2")
                        nc.tensor.transpose(pt2[:], lgT_sb[:, o * P:(o + 1) * P],
                                            ident_f32[:E, :E])
                        nc.vector.tensor_copy(lg_sb[:, ob0 + o], pt2[:])
                nc.sync.dma_start(
                    lg_slice[:].rearrange("(o p) e -> p o e", p=P), lg_sb[:])

            # ---------------- collectives: AllGathers --------------------
            if n_cores > 1:
                nc.gpsimd.collective_compute(
                    "AllGather", ALU.bypass,
                    ins=[lg_slice[:].opt()], outs=[lg_full[:].opt()],
                    replica_groups=[list(range(n_cores))])
            else:
                nc.sync.dma_start(lg_full[:], lg_slice[:])

            for ko in range(KO):
                nc.gpsimd.dma_start(wg_sb[:, ko], wg_r[:, ko])
                nc.gpsimd.dma_start(wu_sb[:, ko], wu_r[:, ko])

            # ---------------- routing math ----------------
            L = main.tile([P, TO, E], dt.float32)
            nc.sync.dma_start(L[:], lg_full[:].rearrange("(o p) e -> p o e", p=P))
            V = main.tile([P, TO, E], dt.float32)
            I = main.tile([P, TO, E], dt.uint32)
            for o in range(TO):
                nc.vector.max(V[:, o], L[:, o])
                nc.vector.max_index(I[:, o], V[:, o], L[:, o])
            m1 = V[:, :, 0]
            m2 = V[:, :, 1]
            If = main.tile([P, TO, 2], dt.float32)
            nc.vector.tensor_copy(If[:], I[:, :, 0:2])

            expL = main.tile([P, TO, E], dt.float32)
            nc.scalar.activation(expL[:], L[:], AF.Exp)
            Z = main.tile([P, TO], dt.float32)
            nc.vector.reduce_sum(Z[:], expL[:], axis=mybir.AxisListType.X)
            E1 = main.tile([P, TO], dt.float32)
            E2 = main.tile([P, TO], dt.float32)
            nc.scalar.activation(E1[:], m1, AF.Exp)
            nc.scalar.activation(E2[:], m2, AF.Exp)
            rZ = main.tile([P, TO], dt.float32)
            nc.vector.reciprocal(rZ[:], Z[:])
            arg = main.tile([P, TO], dt.float32)
            nc.vector.tensor_sub(arg[:], E1[:], E2[:])
            nc.vector.tensor_mul(arg[:], arg[:], rZ[:])
            w1 = main.tile([P, TO], dt.float32)
            nc.scalar.activation(w1[:], arg[:], AF.Sigmoid)

            mask1 = main.tile([P, TO], dt.float32)
            mask2 = main.tile([P, TO], dt.float32)
            nc.vector.tensor_scalar(mask1[:], If[:, :, 0], eid_sb[:], None,
                                    ALU.is_equal)
            nc.vector.tensor_scalar(mask2[:], If[:, :, 1], eid_sb[:], None,
                                    ALU.is_equal)
            mask = main.tile([P, TO], dt.float32)
            nc.vector.tensor_add(mask[:], mask1[:], mask2[:])
            wsel = main.tile([P, TO], dt.float32)
            tmp = main.tile([P, TO], dt.float32, tag="tmp")
            nc.vector.tensor_mul(wsel[:], mask1[:], w1[:])
            nc.vector.tensor_mul(tmp[:], mask2[:], w1[:])
            nc.vector.tensor_add(wsel[:], wsel[:], mask2[:])
            nc.vector.tensor_sub(wsel[:], wsel[:], tmp[:])

            # cumsum down partitions via triangular matmul
            with tc.tile_pool(name="ps_cs", bufs=1, space="PSUM") as pscs_pool:
                maskb = main.tile([P, TO], dt.bfloat16)
                nc.vector.tensor_copy(maskb[:], mask[:])
                ps_cs = pscs_pool.tile([P, TO], dt.float32)
                nc.tensor.matmul(ps_cs[:], triu_bf[:], maskb[:], start=True,
                                 stop=True)
                csum = main.tile([P, TO], dt.float32)
                nc.vector.tensor_copy(csum[:], ps_cs[:])
                ps_tot = pscs_pool.tile([1, TO], dt.float32, name="ps_tot")
                nc.tensor.matmul(ps_tot[:], ones_bf[:], maskb[:], start=True,
                                 stop=True)
                coltot = main.tile([1, TO], dt.float32)
                nc.vector.tensor_copy(coltot[:], ps_tot[:])
            sc_a = main.tile([1, TO], dt.float32, tag="sca")
            sc_b = main.tile([1, TO], dt.float32, tag="scb")
            nc.vector.tensor_copy(sc_a[:], coltot[:])
            cur, nxt = sc_a, sc_b
            s = 1
            while s < TO:
                nc.vector.tensor_copy(nxt[:], cur[:])
                nc.vector.tensor_add(nxt[:, s:], cur[:, s:], cur[:, :TO - s])
                cur, nxt = nxt, cur
                s *= 2
            offs = main.tile([1, TO], dt.float32)
            nc.vector.memset(offs[:, 0:1], 0.0)
            nc.vector.tensor_copy(offs[:, 1:], cur[:, :TO - 1])
            offs_b = main.tile([P, TO], dt.float32)
            nc.gpsimd.partition_broadcast(offs_b[:], offs[:])

            pos = main.tile([P, TO], dt.float32)
            nc.vector.tensor_add(pos[:], csum[:], offs_b[:])
            nc.vector.tensor_sub(pos[:], pos[:], mask[:])
            trashv = main.tile([P, 1], dt.int32)
            nc.gpsimd.iota(trashv[:], pattern=[[0, 1]], base=0,
                           channel_multiplier=1)
            nc.vector.tensor_scalar(trashv[:], trashv[:], 63, None,
                                    ALU.bitwise_and)
            trashf = main.tile([P, 1], dt.float32)
            nc.vector.tensor_copy(trashf[:], trashv[:])
            nc.vector.tensor_scalar(trashf[:], trashf[:], float(C), None, ALU.add)
            slot = main.tile([P, TO], dt.float32)
            nc.vector.tensor_scalar(slot[:], pos[:], trashf[:], None,
                                    ALU.subtract)
            nc.vector.tensor_mul(slot[:], slot[:], mask[:])
            nc.vector.tensor_scalar(slot[:], slot[:], trashf[:], None, ALU.add)
            slot16 = main.tile([P, TO], dt.int16)
            nc.vector.tensor_copy(slot16[:], slot[:])

            # wrapped-by-16 slot list via DRAM roundtrip
            nc.sync.dma_start(slotd[:].rearrange("(o p) -> p o", p=P), slot16[:])
            slot16w = main.tile([P, T // 16], dt.int16)
            slotd_w = slotd[:].rearrange("(cw pw) -> pw cw", pw=16)
            for rep in range(8):
                nc.sync.dma_start(slot16w[rep * 16:(rep + 1) * 16, :], slotd_w)

            # per-token payload rows [hi, lo, wsel, 0...]: 64 f32 = 256B
            tid32 = main.tile([P, TO], dt.int32)
            nc.gpsimd.iota(tid32[:], pattern=[[P, TO]], base=0,
                           channel_multiplier=1)
            hi32 = main.tile([P, TO], dt.int32)
            lo32 = main.tile([P, TO], dt.int32)
            nc.vector.tensor_scalar(hi32[:], tid32[:], 6, None,
                                    ALU.arith_shift_right)
            nc.vector.tensor_scalar(lo32[:], tid32[:], 63, None,
                                    ALU.bitwise_and)
            rows = main.tile([P, TO, 64], dt.float32)
            nc.vector.memset(rows[:], 0.0)
            nc.vector.tensor_copy(rows[:, :, 0], hi32[:])
            nc.vector.tensor_copy(rows[:, :, 1], lo32[:])
            nc.vector.tensor_copy(rows[:, :, 2], wsel[:])
            nc.vector.memset(rows[:, :, 3], 1.0)

            # init ilist2 rows to [64, 0, 0...] (pad slots -> token 4096)
            init_sb = main.tile([P, 64], dt.float32)
            nc.vector.memset(init_sb[:], 0.0)
            nil = C + NTRASH
            for r0 in range(0, nil, P):
                rr = min(P, nil - r0)
                nc.sync.dma_start(ilist2[r0:r0 + rr, :], init_sb[:rr, :])
            # scatter-add the payload rows into slot order
            for k in range(TO // 4):
                nc.gpsimd.dma_scatter_add(
                    out_ap=ilist2[:], in_ap=rows[:, 4 * k:4 * (k + 1), :],
                    idxs_ap=slot16w[:, k * 32:(k + 1) * 32],
                    num_idxs=4 * P, num_idxs_reg=4 * P, elem_size=64)

            # load back: tid + wlist per slot
            lb = main.tile([P, CM, 4], dt.float32)
            nc.sync.dma_start(
                lb[:], ilist2[:C, :4].rearrange("(m p) c -> p m c", p=P))
            tidf = main.tile([P, CM], dt.float32)
            nc.vector.tensor_scalar(tidf[:], lb[:, :, 0], 64.0, None, ALU.mult)
            nc.vector.tensor_add(tidf[:], tidf[:], lb[:, :, 1])
            # empty slots (count==0) -> token T (trash row)
            emptyf = main.tile([P, CM], dt.float32)
            nc.vector.tensor_scalar(emptyf[:], lb[:, :, 3], 1.0, None,
                                    ALU.subtract)
            nc.vector.tensor_scalar(emptyf[:], emptyf[:], float(-T), None,
                                    ALU.mult)
            nc.vector.tensor_add(tidf[:], tidf[:], emptyf[:])
            wlist = main.tile([P, CM], dt.float32)
            nc.vector.tensor_copy(wlist[:], lb[:, :, 2])
            tclf = main.tile([P, CM], dt.float32)
            nc.vector.tensor_scalar(tclf[:], tidf[:], float(T - 1), None,
                                    ALU.min)
            tid16r = main.tile([P, CM], dt.int16)
            tid16c = main.tile([P, CM], dt.int16)
            nc.vector.tensor_copy(tid16r[:], tidf[:])
            nc.vector.tensor_copy(tid16c[:], tclf[:])
            nc.sync.dma_start(tboth[:C].rearrange("(m p) -> p m", p=P),
                              tid16r[:])
            nc.sync.dma_start(tboth[C:].rearrange("(m p) -> p m", p=P),
                              tid16c[:])
            sgi = main.tile([P, 2 * C16], dt.int16)
            tb_w = tboth[:].rearrange("(cw pw) -> pw cw", pw=16)
            for rep in range(8):
                nc.sync.dma_start(sgi[rep * 16:(rep + 1) * 16, :], tb_w)
            si16 = sgi[:, :C16]
            gi16 = sgi[:, C16:]

            nc.vector.memset(zero_sb[:], 0.0)
            for r0 in range(0, T, P):
                nc.sync.dma_start(acc[r0:r0 + P, :], zero_sb[:, :])

            # ---------------- gather + transpose xeT (3 chunks) ----------
            mmp = tc.tile_pool(name="mmp", bufs=1)
            mmpool = mmp.__enter__()
            wd_sb = mmpool.tile([P, FO, D], dt.bfloat16)
            for fo in range(FO):
                nc.gpsimd.dma_start(wd_sb[:, fo], wd_r[:, fo])
            xeTs = [mmpool.tile([P, KO, GC], dt.bfloat16, tag=f"xeT{k}", name=f"xeT{k}")
                    for k in range(3)]
            for k in range(3):
                nc.gpsimd.dma_gather(
                    out_ap=xeTs[k][:], in_ap=xbf_full[:],
                    idxs_ap=sgi[:, C16 + k * (GC // 16):C16 + (k + 1) * (GC // 16)],
                    num_idxs=GC, num_idxs_reg=GC, elem_size=D, transpose=True)

            # ---------------- gate/up matmuls + fuse ----------------
            fuse = mmpool.tile([P, FO, C], dt.bfloat16)
            with tc.tile_pool(name="psgu", bufs=1, space="PSUM") as psgu:
                for fo in range(FO):
                    gbank = [psgu.tile([P, GC], dt.float32, tag=f"g{i}", name=f"g{i}")
                             for i in range(3)]
                    ubank = [psgu.tile([P, GC], dt.float32, tag=f"u{i}", name=f"u{i}")
                             for i in range(3)]
                    for ko in range(KO):
                        st = ko == 0
                        sp = ko == KO - 1
                        for i in range(3):
                            nc.tensor.matmul(gbank[i][:],
                                             wg_sb[:, ko, fo * P:(fo + 1) * P],
                                             xeTs[i][:, ko], start=st, stop=sp)
                        for i in range(3):
                            nc.tensor.matmul(ubank[i][:],
                                             wu_sb[:, ko, fo * P:(fo + 1) * P],
                                             xeTs[i][:, ko], start=st, stop=sp)
                    for i in range(3):
                        sil = mmpool.tile([P, GC], dt.float32, tag="sil")
                        # silu(g)*u = g*sigmoid(g)*u (sim lacks Silu)
                        nc.scalar.activation(sil[:], gbank[i][:], AF.Sigmoid)
                        nc.vector.tensor_mul(sil[:], sil[:], gbank[i][:])
                        nc.vector.tensor_mul(fuse[:, fo, i * GC:(i + 1) * GC],
                                             sil[:], ubank[i][:])

            # ---------------- down matmul + scale + scatter ----------
            dchunks = [(0, 512), (512, 512), (1024, 512), (1536, 512)]
            with tc.tile_pool(name="psd", bufs=1, space="PSUM") as psd, \
                 tc.tile_pool(name="doutp", bufs=3) as doutp:
                for tm in range(CM):
                    dbank = [psd.tile([P, 512], dt.float32, tag=f"d{i}", name=f"d{i}")
                             for i in range(4)]
                    for fo in range(FO):
                        for i, (d0, n) in enumerate(dchunks):
                            nc.tensor.matmul(dbank[i][:],
                                             fuse[:, fo, tm * P:(tm + 1) * P],
                                             wd_sb[:, fo, d0:d0 + n],
                                             start=(fo == 0), stop=(fo == FO - 1))
                    dout = doutp.tile([P, 1, D], dt.bfloat16, tag="dout")
                    for i, (d0, n) in enumerate(dchunks):
                        nc.vector.tensor_scalar(dout[:, 0, d0:d0 + n],
                                                dbank[i][:],
                                                wlist[:, tm:tm + 1], None,
                                                ALU.mult)
                    nc.gpsimd.dma_scatter_add(
                        out_ap=acc[:], in_ap=dout[:],
                        idxs_ap=si16[:, tm * 8:(tm + 1) * 8],
                        num_idxs=P, num_idxs_reg=P, elem_size=D)

            if n_cores > 1:
                nc.gpsimd.collective_compute(
                    "ReduceScatter", ALU.add,
                    ins=[acc[:T].opt()], outs=[rs_out[:].opt()],
                    replica_groups=[list(range(n_cores))])
            else:
                nc.sync.dma_start(rs_out[:], acc[:TS])
            mmp.__exit__(None, None, None)
            rs_r = rs_out[:].rearrange("(o p) d -> p o d", p=P)
            out_r = out[:].rearrange("(o p) d -> p o d", p=P)
            with tc.tile_pool(name="finp", bufs=2) as finp:
                for ob in range(SO):
                    fin = finp.tile([P, D], dt.bfloat16, tag="fin", name="fin")
                    nc.sync.dma_start(fin[:], rs_r[:, ob])
                    fin32 = finp.tile([P, D], dt.float32, tag="fin32",
                                      name="fin32")
                    nc.vector.tensor_copy(fin32[:], fin[:])
                    nc.sync.dma_start(out_r[:, ob], fin32[:])

    nc.compile()
    return nc


_NC_CACHE = {}


def _get_nc():
    if "nc" not in _NC_CACHE:
        _NC_CACHE["nc"] = build(n_cores=8)
    return _NC_CACHE["nc"]


def kernel(x, router_kernel, w_gate, w_up, w_down):
    """Full-input MoE forward on 8 TRN2 NeuronCores (expert-parallel)."""
    from concourse.bass_utils import run_bass_kernel_spmd

    x = np.ascontiguousarray(np.asarray(x, dtype=np.float32))
    rk = np.ascontiguousarray(np.asarray(router_kernel, dtype=np.float32))
    wg = np.asarray(w_gate, dtype=np.float32)
    wu = np.asarray(w_up, dtype=np.float32)
    wd = np.asarray(w_down, dtype=np.float32)

    nc = _get_nc()
    TS = T // 8
    in_maps = []
    for c in range(8):
        in_maps.append({
            "xs": np.ascontiguousarray(x[c * TS:(c + 1) * TS]),
            "rk": rk,
            "wg": np.ascontiguousarray(wg[c]),
            "wu": np.ascontiguousarray(wu[c]),
            "wd": np.ascontiguousarray(wd[c]),
            "eid": np.full((P, 1), float(c), np.float32),
        })
    res = run_bass_kernel_spmd(nc, in_maps, core_ids=list(range(8)))
    out = np.concatenate([res.results[c]["out"] for c in range(8)], axis=0)
    return out.astype(np.float32)

